# revision 13
# baseline (speedup 1.0000x reference)
"""BoundaryFluxAttention TRN2 kernel.

Distribution (8 cores): data-parallel over batch (B=2) x tensor-parallel over
heads (16 heads -> 4 groups of 4). Core c handles batch c//4, head group c%4.
Each core computes a partial output y_c = softmax-attention(its 4 heads) @ W_out
rows for those heads; the host sums the 4 partials per batch and adds b_out.

Per-core pipeline (T=2048, D=1024, 4 heads of hd=64), hand-pipelined emission:
  A:  QK^T projection qkt[db] [128, T] bf16 = (W slice)^T @ xT (fp32r matmuls);
      x arrives pre-transposed from the host. Scale hd^-0.5 folded into W_q/b_q.
  B:  V projection in natural [T, 256] layout -> vsb [128, kb, h, 65] bf16
      with a ones column at index 64 (denominator accumulates in the same
      matmul as O'^T).
  C:  S^T tiles [128k, 1024(2 heads)] = K_h^T.T @ Q_h^T, heads row-tiled
      (K=64 at partition offsets 0/64), bf16: ~113ns/matmul via PE row-group
      concurrency.
  exp: ScalarE, per-partition bias = boundary*0.1, bf16 out. This paces the
      CD loop (~1.1us per k-block) -> emission interleaves stage A/B and the
      previous group's epilogue so ACT never starves.
  D:  O'^T [65, 512] += V'_h.T @ P_h^T over k; row 64 = softmax denominator.
  norm: stage O' to SBUF (frees the PSUM accumulator fast), reciprocal of the
      denominator row, partition-broadcast via K=1 matmul, fused multiply.
      Odd heads shifted to partitions 64..127 via SBUF->SBUF DMA.
  E:  y = OT_pair @ W_out slice (fp32r), emitted per q-block as PE filler.
"""

import numpy as np

import concourse.bass as bass  # noqa: F401
import concourse.mybir as mybir
import concourse.tile as tile
from concourse import bacc

F32 = mybir.dt.float32
F32R = mybir.dt.float32r
BF16 = mybir.dt.bfloat16
EXP = mybir.ActivationFunctionType.Exp

T = 2048
D = 1024
HPC = 4          # heads per core
HD = 64
NKB = T // 128   # 16 k/t blocks of 128
NQB = T // 512   # 4 q blocks of 512
NCH = D // 128   # 8 contraction chunks
SCALE = HD ** -0.5
BIAS_COEF = 0.1

_NC_CACHE = {}


def _build_nc():
    nc = bacc.Bacc("TRN2", target_bir_lowering=False)

    xt_d = nc.declare_dram_parameter("xt", [D, T], F32R, isOutput=False)
    wqk_d = nc.declare_dram_parameter("wqk", [D, 512], F32R, isOutput=False)
    bqk_d = nc.declare_dram_parameter("bqk", [1, 512], F32R, isOutput=False)
    wv_d = nc.declare_dram_parameter("wv", [D, 256], F32R, isOutput=False)
    bv_d = nc.declare_dram_parameter("bv", [1, 256], F32R, isOutput=False)
    wo_d = nc.declare_dram_parameter("wo", [256, D], F32R, isOutput=False)
    bs_d = nc.declare_dram_parameter("bs", [128, NKB], F32, isOutput=False)
    ones_d = nc.declare_dram_parameter("ones", [1, 512], F32R, isOutput=False)
    ones65_d = nc.declare_dram_parameter("ones65", [65, 64], F32R, isOutput=False)
    y_d = nc.declare_dram_parameter("y", [T, D], F32, isOutput=True)

    with tile.TileContext(nc) as tc:
        with (
            tc.tile_pool(name="const", bufs=1) as constp,
            tc.tile_pool(name="wts", bufs=1) as wts,
            tc.tile_pool(name="big", bufs=1) as bigp,
            tc.tile_pool(name="pt", bufs=4) as ptp,
            tc.tile_pool(name="norm", bufs=1) as normp,
            tc.tile_pool(name="stg", bufs=2) as stgp,
            tc.tile_pool(name="ysb", bufs=3) as ypool,
            tc.tile_pool(name="psG", bufs=2, space="PSUM") as psG,
            tc.tile_pool(name="psS", bufs=2, space="PSUM") as psS,
            tc.tile_pool(name="psO", bufs=1, space="PSUM") as psO,
        ):
            # ---------------- constants / weights (scalar DMA queue) --------
            ones = constp.tile([1, 512], F32R, tag="ones")
            nc.scalar.dma_start(ones[:], ones_d[:])
            ones65 = constp.tile([65, 64], F32R, tag="ones65")
            nc.scalar.dma_start(ones65[:], ones65_d[:])
            bs_sb = constp.tile([128, NKB], F32, tag="bs")
            nc.scalar.dma_start(bs_sb[:], bs_d[:])

            wqk_sb = wts.tile([128, NCH, 512], F32R, tag="wqk")
            nc.scalar.dma_start(wqk_sb[:], wqk_d.rearrange("(c p) n -> p c n", p=128))
            wv_sb = wts.tile([128, NCH, 256], F32R, tag="wv")
            nc.scalar.dma_start(wv_sb[:], wv_d.rearrange("(c p) n -> p c n", p=128))
            wo_sb = wts.tile([128, 2, D], F32R, tag="wo")
            nc.scalar.dma_start(wo_sb[:], wo_d.rearrange("(c p) n -> p c n", p=128))
            bqk_sb = wts.tile([1, 512], F32R, tag="bqk")
            nc.scalar.dma_start(bqk_sb[:], bqk_d[:])
            bv_sb = wts.tile([1, 256], F32R, tag="bv")
            nc.scalar.dma_start(bv_sb[:], bv_d[:])

            # x^T chunks on the sync queue so they land first
            xT = bigp.tile([128, NCH, T], F32R, tag="xT")
            xt_chunks = xt_d.rearrange("(c p) t -> c p t", p=128)
            for c in range(NCH):
                nc.sync.dma_start(xT[:, c, :], xt_chunks[c])

            qkt = [
                bigp.tile([128, T], BF16, tag=f"qkt{db}", name=f"qkt{db}")
                for db in range(4)
            ]
            vsb = bigp.tile([128, NKB, HPC, 65], BF16, tag="vsb")
            nc.vector.memset(vsb[:], 1.0)
            ot = [
                bigp.tile([128, T], F32R, tag=f"ot{pi}", name=f"ot{pi}")
                for pi in range(2)
            ]
            y_rows = y_d.rearrange("(n p) d -> n p d", p=128)

            # ---------------- emission helpers ----------------
            def emit_A(tb):
                for db in range(4):
                    ps = psG.tile([128, 512], F32, tag="gp", name=f"qk{tb}_{db}")
                    for c in range(NCH):
                        nc.tensor.matmul(
                            ps[:],
                            wqk_sb[:, c, db * 128:(db + 1) * 128],
                            xT[:, c, tb * 512:(tb + 1) * 512],
                            start=(c == 0),
                            stop=False,
                        )
                    nc.tensor.matmul(
                        ps[:],
                        bqk_sb[0:1, db * 128:(db + 1) * 128],
                        ones[0:1, :],
                        start=False,
                        stop=True,
                    )
                    nc.vector.tensor_copy(qkt[db][:, tb * 512:(tb + 1) * 512], ps[:])

            def emit_B(tb):
                for j in range(4):
                    kb = tb * 4 + j
                    ps = psG.tile([128, 256], F32, tag="gp", name=f"v{kb}")
                    for c in range(NCH):
                        nc.tensor.matmul(
                            ps[:],
                            xT[:, c, kb * 128:(kb + 1) * 128],
                            wv_sb[:, c, :],
                            start=(c == 0),
                            stop=False,
                        )
                    nc.tensor.matmul(
                        ps[:], ones[0:1, 0:128], bv_sb[:], start=False, stop=True
                    )
                    nc.vector.tensor_copy(
                        vsb[:, kb, :, 0:64],
                        ps[:].rearrange("p (h c) -> p h c", h=HPC),
                    )

            def emit_S_exp(qb, pi, kb):
                qdb, kdb = pi, 2 + pi
                s01 = psS.tile([128, 1024], F32, tag="s01", name=f"s{qb}_{pi}_{kb}")
                nc.tensor.matmul(
                    s01[:, 0:512],
                    qkt[kdb][0:64, kb * 128:(kb + 1) * 128],
                    qkt[qdb][0:64, qb * 512:(qb + 1) * 512],
                )
                nc.tensor.matmul(
                    s01[:, 512:1024],
                    qkt[kdb][64:128, kb * 128:(kb + 1) * 128],
                    qkt[qdb][64:128, qb * 512:(qb + 1) * 512],
                )
                p01 = ptp.tile([128, 1024], BF16, tag="p01", name=f"p{qb}_{pi}_{kb}")
                nc.scalar.activation(p01[:], s01[:], EXP, bias=bs_sb[:, kb:kb + 1])
                return p01

            def emit_D(qb, pi, kb, p01, osA, osB):
                nc.tensor.matmul(
                    osA[:], vsb[:, kb, 2 * pi, :], p01[:, 0:512],
                    start=(kb == 0), stop=(kb == NKB - 1),
                )
                nc.tensor.matmul(
                    osB[:], vsb[:, kb, 2 * pi + 1, :], p01[:, 512:1024],
                    start=(kb == 0), stop=(kb == NKB - 1),
                )

            def emit_post(qb, pi, osA, osB):
                # stage O' out of PSUM fast, then normalize from the staging copy
                for parity, os_ps in ((0, osA), (1, osB)):
                    stg = stgp.tile(
                        [65, 512], F32, tag=f"stg{parity}", name=f"stg{qb}_{pi}_{parity}"
                    )
                    nc.vector.tensor_copy(stg[:], os_ps[:])
                    rec = normp.tile([65, 512], F32R, tag=f"rec{parity}")
                    with nc.allow_low_precision(reason="float32r is fp32-width"):
                        nc.vector.reciprocal(rec[64:65, :], stg[64:65, :])
                    bc_ps = psG.tile(
                        [64, 512], F32, tag="gp", name=f"bc{qb}_{pi}_{parity}"
                    )
                    nc.tensor.matmul(bc_ps[:], ones65[64:65, 0:64], rec[64:65, :])
                    cols = slice(qb * 512, (qb + 1) * 512)
                    if parity == 0:
                        nc.vector.tensor_mul(
                            ot[pi][0:64, cols], stg[0:64, :], bc_ps[0:64, :]
                        )
                    else:
                        stag = normp.tile([64, 512], F32R, tag="stag")
                        nc.vector.tensor_mul(stag[:], stg[0:64, :], bc_ps[0:64, :])
                        nc.sync.dma_start(ot[pi][64:128, cols], stag[:])

            def make_E_chunks(qb):
                # Stage E for one q-block, sliced into 16 small closures so the
                # emission can interleave one chunk per k-iteration of the next
                # attention group (keeps PE dense without starving ScalarE).
                chunks = []
                for j in range(4):
                    tb = qb * 4 + j
                    state = {}

                    def c0(tb=tb, state=state):
                        state["ysb"] = ypool.tile(
                            [128, D], F32, tag="ysb", name=f"ysb{tb}"
                        )
                        state["yps"] = [
                            psG.tile([128, 512], F32, tag="gp", name=f"yps{tb}_{nb}")
                            for nb in range(2)
                        ]
                        nc.tensor.matmul(
                            state["yps"][0][:],
                            ot[0][:, tb * 128:(tb + 1) * 128],
                            wo_sb[:, 0, 0:512],
                            start=True, stop=False,
                        )

                    def c1(tb=tb, state=state):
                        nc.tensor.matmul(
                            state["yps"][1][:],
                            ot[0][:, tb * 128:(tb + 1) * 128],
                            wo_sb[:, 0, 512:1024],
                            start=True, stop=False,
                        )

                    def c2(tb=tb, state=state):
                        nc.tensor.matmul(
                            state["yps"][0][:],
                            ot[1][:, tb * 128:(tb + 1) * 128],
                            wo_sb[:, 1, 0:512],
                            start=False, stop=True,
                        )
                        nc.vector.tensor_copy(
                            state["ysb"][:, 0:512], state["yps"][0][:]
                        )

                    def c3(tb=tb, state=state):
                        nc.tensor.matmul(
                            state["yps"][1][:],
                            ot[1][:, tb * 128:(tb + 1) * 128],
                            wo_sb[:, 1, 512:1024],
                            start=False, stop=True,
                        )
                        nc.vector.tensor_copy(
                            state["ysb"][:, 512:1024], state["yps"][1][:]
                        )
                        nc.sync.dma_start(y_rows[tb], state["ysb"][:])

                    chunks += [c0, c1, c2, c3]
                return chunks

            def emit_E(qb):
                for ch in make_E_chunks(qb):
                    ch()

            # ---------------- pipelined emission ----------------
            # Phase 1: stages A/B per t-group, with CD(q0, pair0) k-iterations
            # interleaved so ScalarE ramps while the PE grinds projections.
            osA = psO.tile([65, 512], F32, tag="osA", name="osA0_0")
            osB = psO.tile([65, 512], F32, tag="osB", name="osB0_0")
            for tb in range(4):
                emit_A(tb)
                emit_B(tb)
                for kb in range(4 * tb, 4 * tb + 4):
                    p01 = emit_S_exp(0, 0, kb)
                    emit_D(0, 0, kb, p01, osA, osB)
            pending = [(0, 0, osA, osB)]

            # Phase 2: remaining groups; each group's first two S/exp pairs
            # are emitted before the previous group's epilogue so ACT stays fed
            # across the boundary.
            groups = [(0, 1)] + [(qb, pi) for qb in range(1, NQB) for pi in range(2)]
            e_chunks = []
            for qb, pi in groups:
                head = [emit_S_exp(qb, pi, kb) for kb in (0, 1)]
                # previous group's epilogue; its E work is sliced into chunks
                # and interleaved below
                pqb, ppi, posA, posB = pending.pop()
                emit_post(pqb, ppi, posA, posB)
                if ppi == 1:
                    e_chunks = make_E_chunks(pqb)
                osA = psO.tile([65, 512], F32, tag="osA", name=f"osA{qb}_{pi}")
                osB = psO.tile([65, 512], F32, tag="osB", name=f"osB{qb}_{pi}")
                for kb in (0, 1):
                    emit_D(qb, pi, kb, head[kb], osA, osB)
                for kb in range(2, NKB):
                    p01 = emit_S_exp(qb, pi, kb)
                    emit_D(qb, pi, kb, p01, osA, osB)
                    if e_chunks:
                        e_chunks.pop(0)()
                pending = [(qb, pi, osA, osB)]
                while e_chunks:
                    e_chunks.pop(0)()

            qb, pi, osA, osB = pending.pop()
            emit_post(qb, pi, osA, osB)
            emit_E(qb)

    nc.compile()
    return nc


def _get_nc():
    if "nc" not in _NC_CACHE:
        _NC_CACHE["nc"] = _build_nc()
    return _NC_CACHE["nc"]


def _make_in_maps(x, boundary_score, W_qkv, b_qkv, W_out):
    x = np.asarray(x, np.float32)
    boundary_score = np.asarray(boundary_score, np.float32)
    W_qkv = np.asarray(W_qkv, np.float32)
    b_qkv = np.asarray(b_qkv, np.float32)
    W_out = np.asarray(W_out, np.float32)

    Wq, Wk, Wv = W_qkv[:, :D], W_qkv[:, D:2 * D], W_qkv[:, 2 * D:]
    bq, bk, bv = b_qkv[:D], b_qkv[D:2 * D], b_qkv[2 * D:]
    ones = np.ones((1, 512), np.float32)
    ones65 = np.ones((65, 64), np.float32)
    xts = [np.ascontiguousarray(x[b].T) for b in range(x.shape[0])]

    in_maps = []
    for c in range(8):
        b, g = divmod(c, 4)
        lo, hi = 256 * g, 256 * (g + 1)
        wqk = np.ascontiguousarray(
            np.concatenate([Wq[:, lo:hi] * SCALE, Wk[:, lo:hi]], axis=1)
        )
        bqk = np.concatenate([bq[lo:hi] * SCALE, bk[lo:hi]])[None]
        wv = np.ascontiguousarray(Wv[:, lo:hi])
        bvv = np.ascontiguousarray(bv[lo:hi][None])
        wo = np.ascontiguousarray(W_out[lo:hi, :])
        bs = np.ascontiguousarray(
            (boundary_score[b] * BIAS_COEF).reshape(NKB, 128).T
        )
        in_maps.append(
            dict(
                xt=xts[b], wqk=wqk, bqk=np.ascontiguousarray(bqk),
                wv=wv, bv=bvv, wo=wo, bs=bs, ones=ones, ones65=ones65,
            )
        )
    return in_maps


def kernel(x, boundary_score, W_qkv, b_qkv, W_out, b_out):
    from concourse.bass_utils import run_bass_kernel_spmd

    x = np.asarray(x, np.float32)
    B = x.shape[0]
    in_maps = _make_in_maps(x, boundary_score, W_qkv, b_qkv, W_out)
    nc = _get_nc()
    res = run_bass_kernel_spmd(nc, in_maps, list(range(8))).results
    out = np.zeros((B, T, D), np.float32)
    for c in range(8):
        out[c // 4] += res[c]["y"]
    out += np.asarray(b_out, np.float32)
    return out


# revision 15
# speedup vs baseline: 1.0008x; 1.0008x over previous
"""BoundaryFluxAttention TRN2 kernel.

Distribution (8 cores): data-parallel over batch (B=2) x tensor-parallel over
heads (16 heads -> 4 groups of 4). Core c handles batch c//4, head group c%4.
Each core computes a partial output y_c = softmax-attention(its 4 heads) @ W_out
rows for those heads; the host sums the 4 partials per batch and adds b_out.

Per-core pipeline (T=2048, D=1024, 4 heads of hd=64), hand-pipelined emission:
  A:  QK^T projection qkt[db] [128, T] bf16 = (W slice)^T @ xT (fp32r matmuls);
      x arrives pre-transposed from the host. Scale hd^-0.5 folded into W_q/b_q.
  B:  V projection in natural [T, 256] layout -> vsb [128, kb, h, 65] bf16
      with a ones column at index 64 (denominator accumulates in the same
      matmul as O'^T).
  C:  S^T tiles [128k, 1024(2 heads)] = K_h^T.T @ Q_h^T, heads row-tiled
      (K=64 at partition offsets 0/64), bf16: ~113ns/matmul via PE row-group
      concurrency.
  exp: ScalarE, per-partition bias = boundary*0.1, bf16 out. This paces the
      CD loop (~1.1us per k-block) -> emission interleaves stage A/B and the
      previous group's epilogue so ACT never starves.
  D:  O'^T [65, 512] += V'_h.T @ P_h^T over k; row 64 = softmax denominator.
  norm: stage O' to SBUF (frees the PSUM accumulator fast), reciprocal of the
      denominator row, partition-broadcast via K=1 matmul, fused multiply.
      Odd heads shifted to partitions 64..127 via SBUF->SBUF DMA.
  E:  y = OT_pair @ W_out slice (fp32r), emitted per q-block as PE filler.
"""

import numpy as np

import concourse.bass as bass  # noqa: F401
import concourse.mybir as mybir
import concourse.tile as tile
from concourse import bacc

F32 = mybir.dt.float32
F32R = mybir.dt.float32r
BF16 = mybir.dt.bfloat16
EXP = mybir.ActivationFunctionType.Exp

T = 2048
D = 1024
HPC = 4          # heads per core
HD = 64
NKB = T // 128   # 16 k/t blocks of 128
NQB = T // 512   # 4 q blocks of 512
NCH = D // 128   # 8 contraction chunks
SCALE = HD ** -0.5
BIAS_COEF = 0.1

_NC_CACHE = {}


def _build_nc():
    nc = bacc.Bacc("TRN2", target_bir_lowering=False)

    xt_d = nc.declare_dram_parameter("xt", [D, T], F32R, isOutput=False)
    wqk_d = nc.declare_dram_parameter("wqk", [D, 512], F32R, isOutput=False)
    bqk_d = nc.declare_dram_parameter("bqk", [1, 512], F32R, isOutput=False)
    wv_d = nc.declare_dram_parameter("wv", [D, 256], F32R, isOutput=False)
    bv_d = nc.declare_dram_parameter("bv", [1, 256], F32R, isOutput=False)
    wo_d = nc.declare_dram_parameter("wo", [256, D], F32R, isOutput=False)
    bs_d = nc.declare_dram_parameter("bs", [128, NKB], F32, isOutput=False)
    ones_d = nc.declare_dram_parameter("ones", [1, 512], F32R, isOutput=False)
    ones65_d = nc.declare_dram_parameter("ones65", [65, 64], F32R, isOutput=False)
    idn_d = nc.declare_dram_parameter("idn", [128, 128], F32, isOutput=False)
    y_d = nc.declare_dram_parameter("y", [T, D], F32, isOutput=True)

    with tile.TileContext(nc) as tc:
        with (
            tc.tile_pool(name="const", bufs=1) as constp,
            tc.tile_pool(name="wts", bufs=1) as wts,
            tc.tile_pool(name="big", bufs=1) as bigp,
            tc.tile_pool(name="pt", bufs=4) as ptp,
            tc.tile_pool(name="norm", bufs=1) as normp,
            tc.tile_pool(name="stg", bufs=2) as stgp,
            tc.tile_pool(name="ysb", bufs=3) as ypool,
            tc.tile_pool(name="psG", bufs=2, space="PSUM") as psG,
            tc.tile_pool(name="psS", bufs=2, space="PSUM") as psS,
            tc.tile_pool(name="psO", bufs=1, space="PSUM") as psO,
        ):
            # ---------------- constants / weights (scalar DMA queue) --------
            ones = constp.tile([1, 512], F32R, tag="ones")
            nc.scalar.dma_start(ones[:], ones_d[:])
            ones65 = constp.tile([65, 64], F32R, tag="ones65")
            nc.scalar.dma_start(ones65[:], ones65_d[:])
            ones65f = constp.tile([65, 64], F32, tag="ones65f")
            nc.gpsimd.dma_start(ones65f[:], ones65_d[:])
            idn = constp.tile([128, 128], F32, tag="idn")
            nc.scalar.dma_start(idn[:], idn_d[:])
            bs_sb = constp.tile([128, NKB], F32, tag="bs")
            nc.scalar.dma_start(bs_sb[:], bs_d[:])

            wqk_sb = wts.tile([128, NCH, 512], F32R, tag="wqk")
            nc.scalar.dma_start(wqk_sb[:], wqk_d.rearrange("(c p) n -> p c n", p=128))
            wv_sb = wts.tile([128, NCH, 256], F32R, tag="wv")
            nc.scalar.dma_start(wv_sb[:], wv_d.rearrange("(c p) n -> p c n", p=128))
            wo_sb = wts.tile([128, 2, D], F32R, tag="wo")
            nc.scalar.dma_start(wo_sb[:], wo_d.rearrange("(c p) n -> p c n", p=128))
            bqk_sb = wts.tile([1, 512], F32R, tag="bqk")
            nc.scalar.dma_start(bqk_sb[:], bqk_d[:])
            bv_sb = wts.tile([1, 256], F32R, tag="bv")
            nc.scalar.dma_start(bv_sb[:], bv_d[:])

            # x^T chunks on the sync queue so they land first
            xT = bigp.tile([128, NCH, T], F32R, tag="xT")
            xt_chunks = xt_d.rearrange("(c p) t -> c p t", p=128)
            for tb in range(4):
                for c in range(NCH):
                    nc.sync.dma_start(
                        xT[:, c, tb * 512:(tb + 1) * 512],
                        xt_chunks[c][:, tb * 512:(tb + 1) * 512],
                    )

            qkt = [
                bigp.tile([128, T], BF16, tag=f"qkt{db}", name=f"qkt{db}")
                for db in range(4)
            ]
            vsb = bigp.tile([128, NKB, HPC, 65], BF16, tag="vsb")
            nc.vector.memset(vsb[:], 1.0)
            ot = [
                bigp.tile([128, T], F32R, tag=f"ot{pi}", name=f"ot{pi}")
                for pi in range(2)
            ]
            y_rows = y_d.rearrange("(n p) d -> n p d", p=128)

            # ---------------- emission helpers ----------------
            def emit_A(tb):
                for db in range(4):
                    ps = psG.tile([128, 512], F32, tag="gp", name=f"qk{tb}_{db}")
                    for c in range(NCH):
                        nc.tensor.matmul(
                            ps[:],
                            wqk_sb[:, c, db * 128:(db + 1) * 128],
                            xT[:, c, tb * 512:(tb + 1) * 512],
                            start=(c == 0),
                            stop=False,
                        )
                    nc.tensor.matmul(
                        ps[:],
                        bqk_sb[0:1, db * 128:(db + 1) * 128],
                        ones[0:1, :],
                        start=False,
                        stop=True,
                    )
                    nc.vector.tensor_copy(qkt[db][:, tb * 512:(tb + 1) * 512], ps[:])

            def emit_B(tb):
                for j in range(4):
                    kb = tb * 4 + j
                    ps = psG.tile([128, 256], F32, tag="gp", name=f"v{kb}")
                    for c in range(NCH):
                        nc.tensor.matmul(
                            ps[:],
                            xT[:, c, kb * 128:(kb + 1) * 128],
                            wv_sb[:, c, :],
                            start=(c == 0),
                            stop=False,
                        )
                    nc.tensor.matmul(
                        ps[:], ones[0:1, 0:128], bv_sb[:], start=False, stop=True
                    )
                    nc.vector.tensor_copy(
                        vsb[:, kb, :, 0:64],
                        ps[:].rearrange("p (h c) -> p h c", h=HPC),
                    )

            def emit_S_exp(qb, pi, kb):
                qdb, kdb = pi, 2 + pi
                s01 = psS.tile([128, 1024], F32, tag="s01", name=f"s{qb}_{pi}_{kb}")
                nc.tensor.matmul(
                    s01[:, 0:512],
                    qkt[kdb][0:64, kb * 128:(kb + 1) * 128],
                    qkt[qdb][0:64, qb * 512:(qb + 1) * 512],
                )
                nc.tensor.matmul(
                    s01[:, 512:1024],
                    qkt[kdb][64:128, kb * 128:(kb + 1) * 128],
                    qkt[qdb][64:128, qb * 512:(qb + 1) * 512],
                )
                p01 = ptp.tile([128, 1024], BF16, tag="p01", name=f"p{qb}_{pi}_{kb}")
                nc.scalar.activation(p01[:], s01[:], EXP, bias=bs_sb[:, kb:kb + 1])
                return p01

            def emit_D(qb, pi, kb, p01, osA, osB):
                nc.tensor.matmul(
                    osA[:], vsb[:, kb, 2 * pi, :], p01[:, 0:512],
                    start=(kb == 0), stop=(kb == NKB - 1),
                )
                nc.tensor.matmul(
                    osB[:], vsb[:, kb, 2 * pi + 1, :], p01[:, 512:1024],
                    start=(kb == 0), stop=(kb == NKB - 1),
                )

            def emit_post(qb, pi, osA, osB):
                # stage O' out of PSUM fast, then normalize from the staging copy
                for parity, os_ps in ((0, osA), (1, osB)):
                    stg = stgp.tile(
                        [65, 512], F32, tag=f"stg{parity}", name=f"stg{qb}_{pi}_{parity}"
                    )
                    nc.vector.tensor_copy(stg[:], os_ps[:])
                    # spread the denominator row across 128 partitions so the
                    # iterative-divide reciprocal runs at FD=4 instead of 512
                    dn_ps = psG.tile(
                        [128, 4], F32, tag="gp", name=f"dn{qb}_{pi}_{parity}"
                    )
                    for xx in range(4):
                        nc.tensor.matmul(
                            dn_ps[:, xx:xx + 1],
                            stg[64:65, xx * 128:(xx + 1) * 128],
                            ones65f[64:65, 0:1],
                            skip_group_check=True,
                        )
                    recw = normp.tile([128, 4], F32, tag=f"recw{parity}")
                    nc.vector.reciprocal(recw[:], dn_ps[:])
                    rr_ps = psG.tile(
                        [1, 512], F32, tag="gp", name=f"rr{qb}_{pi}_{parity}"
                    )
                    for xx in range(4):
                        nc.tensor.matmul(
                            rr_ps[0:1, xx * 128:(xx + 1) * 128],
                            recw[:, xx:xx + 1],
                            idn[:],
                            skip_group_check=True,
                        )
                    rec = normp.tile([65, 512], F32, tag=f"rec{parity}")
                    nc.vector.tensor_copy(rec[0:1, :], rr_ps[0:1, :])
                    bc_ps = psG.tile(
                        [64, 512], F32, tag="gp", name=f"bc{qb}_{pi}_{parity}"
                    )
                    nc.tensor.matmul(bc_ps[:], ones65f[0:1, 0:64], rec[0:1, :])
                    cols = slice(qb * 512, (qb + 1) * 512)
                    if parity == 0:
                        nc.vector.tensor_mul(
                            ot[pi][0:64, cols], stg[0:64, :], bc_ps[0:64, :]
                        )
                    else:
                        stag = normp.tile([64, 512], F32R, tag="stag")
                        nc.vector.tensor_mul(stag[:], stg[0:64, :], bc_ps[0:64, :])
                        nc.sync.dma_start(ot[pi][64:128, cols], stag[:])

            def make_E_chunks(qb):
                # Stage E for one q-block, sliced into 16 small closures so the
                # emission can interleave one chunk per k-iteration of the next
                # attention group (keeps PE dense without starving ScalarE).
                chunks = []
                for j in range(4):
                    tb = qb * 4 + j
                    state = {}

                    def c0(tb=tb, state=state):
                        state["ysb"] = ypool.tile(
                            [128, D], F32, tag="ysb", name=f"ysb{tb}"
                        )
                        state["yps"] = [
                            psG.tile([128, 512], F32, tag="gp", name=f"yps{tb}_{nb}")
                            for nb in range(2)
                        ]
                        nc.tensor.matmul(
                            state["yps"][0][:],
                            ot[0][:, tb * 128:(tb + 1) * 128],
                            wo_sb[:, 0, 0:512],
                            start=True, stop=False,
                        )

                    def c1(tb=tb, state=state):
                        nc.tensor.matmul(
                            state["yps"][1][:],
                            ot[0][:, tb * 128:(tb + 1) * 128],
                            wo_sb[:, 0, 512:1024],
                            start=True, stop=False,
                        )

                    def c2(tb=tb, state=state):
                        nc.tensor.matmul(
                            state["yps"][0][:],
                            ot[1][:, tb * 128:(tb + 1) * 128],
                            wo_sb[:, 1, 0:512],
                            start=False, stop=True,
                        )
                        nc.vector.tensor_copy(
                            state["ysb"][:, 0:512], state["yps"][0][:]
                        )

                    def c3(tb=tb, state=state):
                        nc.tensor.matmul(
                            state["yps"][1][:],
                            ot[1][:, tb * 128:(tb + 1) * 128],
                            wo_sb[:, 1, 512:1024],
                            start=False, stop=True,
                        )
                        nc.vector.tensor_copy(
                            state["ysb"][:, 512:1024], state["yps"][1][:]
                        )
                        nc.sync.dma_start(y_rows[tb], state["ysb"][:])

                    chunks += [c0, c1, c2, c3]
                return chunks

            def emit_E(qb):
                for ch in make_E_chunks(qb):
                    ch()

            # ---------------- pipelined emission ----------------
            # Phase 1: stages A/B per t-group, with CD(q0, pair0) k-iterations
            # interleaved so ScalarE ramps while the PE grinds projections.
            osA = psO.tile([65, 512], F32, tag="osA", name="osA0_0")
            osB = psO.tile([65, 512], F32, tag="osB", name="osB0_0")
            for tb in range(4):
                emit_A(tb)
                emit_B(tb)
                for kb in range(4 * tb, 4 * tb + 4):
                    p01 = emit_S_exp(0, 0, kb)
                    emit_D(0, 0, kb, p01, osA, osB)
            pending = [(0, 0, osA, osB)]

            # Phase 2: remaining groups; each group's first two S/exp pairs
            # are emitted before the previous group's epilogue so ACT stays fed
            # across the boundary.
            groups = [(0, 1)] + [(qb, pi) for qb in range(1, NQB) for pi in range(2)]
            e_chunks = []
            for qb, pi in groups:
                head = [emit_S_exp(qb, pi, kb) for kb in (0, 1)]
                # previous group's epilogue; its E work is sliced into chunks
                # and interleaved below
                pqb, ppi, posA, posB = pending.pop()
                emit_post(pqb, ppi, posA, posB)
                if ppi == 1:
                    e_chunks = make_E_chunks(pqb)
                osA = psO.tile([65, 512], F32, tag="osA", name=f"osA{qb}_{pi}")
                osB = psO.tile([65, 512], F32, tag="osB", name=f"osB{qb}_{pi}")
                for kb in (0, 1):
                    emit_D(qb, pi, kb, head[kb], osA, osB)
                for kb in range(2, NKB):
                    p01 = emit_S_exp(qb, pi, kb)
                    emit_D(qb, pi, kb, p01, osA, osB)
                    if e_chunks:
                        e_chunks.pop(0)()
                pending = [(qb, pi, osA, osB)]
                while e_chunks:
                    e_chunks.pop(0)()

            qb, pi, osA, osB = pending.pop()
            emit_post(qb, pi, osA, osB)
            emit_E(qb)

    nc.compile()
    return nc


def _get_nc():
    if "nc" not in _NC_CACHE:
        _NC_CACHE["nc"] = _build_nc()
    return _NC_CACHE["nc"]


def _make_in_maps(x, boundary_score, W_qkv, b_qkv, W_out):
    x = np.asarray(x, np.float32)
    boundary_score = np.asarray(boundary_score, np.float32)
    W_qkv = np.asarray(W_qkv, np.float32)
    b_qkv = np.asarray(b_qkv, np.float32)
    W_out = np.asarray(W_out, np.float32)

    Wq, Wk, Wv = W_qkv[:, :D], W_qkv[:, D:2 * D], W_qkv[:, 2 * D:]
    bq, bk, bv = b_qkv[:D], b_qkv[D:2 * D], b_qkv[2 * D:]
    ones = np.ones((1, 512), np.float32)
    ones65 = np.ones((65, 64), np.float32)
    xts = [np.ascontiguousarray(x[b].T) for b in range(x.shape[0])]
    idn = np.eye(128, dtype=np.float32)

    in_maps = []
    for c in range(8):
        b, g = divmod(c, 4)
        lo, hi = 256 * g, 256 * (g + 1)
        wqk = np.ascontiguousarray(
            np.concatenate([Wq[:, lo:hi] * SCALE, Wk[:, lo:hi]], axis=1)
        )
        bqk = np.concatenate([bq[lo:hi] * SCALE, bk[lo:hi]])[None]
        wv = np.ascontiguousarray(Wv[:, lo:hi])
        bvv = np.ascontiguousarray(bv[lo:hi][None])
        wo = np.ascontiguousarray(W_out[lo:hi, :])
        bs = np.ascontiguousarray(
            (boundary_score[b] * BIAS_COEF).reshape(NKB, 128).T
        )
        in_maps.append(
            dict(
                xt=xts[b], wqk=wqk, bqk=np.ascontiguousarray(bqk),
                wv=wv, bv=bvv, wo=wo, bs=bs, ones=ones, ones65=ones65, idn=idn,
            )
        )
    return in_maps


def kernel(x, boundary_score, W_qkv, b_qkv, W_out, b_out):
    from concourse.bass_utils import run_bass_kernel_spmd

    x = np.asarray(x, np.float32)
    B = x.shape[0]
    in_maps = _make_in_maps(x, boundary_score, W_qkv, b_qkv, W_out)
    nc = _get_nc()
    res = run_bass_kernel_spmd(nc, in_maps, list(range(8))).results
    out = np.zeros((B, T, D), np.float32)
    for c in range(8):
        out[c // 4] += res[c]["y"]
    out += np.asarray(b_out, np.float32)
    return out


# revision 17
# speedup vs baseline: 1.0920x; 1.0911x over previous
"""BoundaryFluxAttention TRN2 kernel.

Distribution (8 cores): data-parallel over batch (B=2) x tensor-parallel over
heads (16 heads -> 4 groups of 4). Core c handles batch c//4, head group c%4.
Each core computes a partial output y_c = softmax-attention(its 4 heads) @ W_out
rows for those heads; the host sums the 4 partials per batch and adds b_out.

Per-core pipeline (T=2048, D=1024, 4 heads of hd=64), hand-pipelined emission:
  A:  QK^T projection qkt[db] [128, T] bf16 = (W slice)^T @ xT (fp32r matmuls);
      x arrives pre-transposed from the host. Scale hd^-0.5 folded into W_q/b_q.
  B:  V projection in natural [T, 256] layout -> vsb [128, kb, h, 65] bf16
      with a ones column at index 64 (denominator accumulates in the same
      matmul as O'^T).
  C:  S^T tiles [128k, 1024(2 heads)] = K_h^T.T @ Q_h^T, heads row-tiled
      (K=64 at partition offsets 0/64), bf16: ~113ns/matmul via PE row-group
      concurrency.
  exp: ScalarE, per-partition bias = boundary*0.1, bf16 out. This paces the
      CD loop (~1.1us per k-block) -> emission interleaves stage A/B and the
      previous group's epilogue so ACT never starves.
  D:  O'^T [65, 512] += V'_h.T @ P_h^T over k; row 64 = softmax denominator.
  norm: stage O' to SBUF (frees the PSUM accumulator fast), reciprocal of the
      denominator row, partition-broadcast via K=1 matmul, fused multiply.
      Odd heads shifted to partitions 64..127 via SBUF->SBUF DMA.
  E:  y = OT_pair @ W_out slice (fp32r), emitted per q-block as PE filler.
"""

import numpy as np

import concourse.bass as bass  # noqa: F401
import concourse.mybir as mybir
import concourse.tile as tile
from concourse import bacc

F32 = mybir.dt.float32
F32R = mybir.dt.float32r
BF16 = mybir.dt.bfloat16
EXP = mybir.ActivationFunctionType.Exp

T = 2048
D = 1024
HPC = 4          # heads per core
HD = 64
NKB = T // 128   # 16 k/t blocks of 128
NQB = T // 512   # 4 q blocks of 512
NCH = D // 128   # 8 contraction chunks
SCALE = HD ** -0.5
BIAS_COEF = 0.1

_NC_CACHE = {}


def _build_nc():
    nc = bacc.Bacc("TRN2", target_bir_lowering=False)

    xt_d = nc.declare_dram_parameter("xt", [D, T], F32R, isOutput=False)
    wqk_d = nc.declare_dram_parameter("wqk", [D, 512], F32R, isOutput=False)
    bqk_d = nc.declare_dram_parameter("bqk", [1, 512], F32R, isOutput=False)
    wv_d = nc.declare_dram_parameter("wv", [D, 256], F32R, isOutput=False)
    bv_d = nc.declare_dram_parameter("bv", [1, 256], F32R, isOutput=False)
    wo_d = nc.declare_dram_parameter("wo", [256, D], F32R, isOutput=False)
    bs_d = nc.declare_dram_parameter("bs", [128, NKB], F32, isOutput=False)
    ones_d = nc.declare_dram_parameter("ones", [1, 512], F32R, isOutput=False)
    ones65_d = nc.declare_dram_parameter("ones65", [65, 64], F32R, isOutput=False)
    y_d = nc.declare_dram_parameter("y", [T, D], F32, isOutput=True)

    with tile.TileContext(nc) as tc:
        with (
            tc.tile_pool(name="const", bufs=1) as constp,
            tc.tile_pool(name="wts", bufs=1) as wts,
            tc.tile_pool(name="big", bufs=1) as bigp,
            tc.tile_pool(name="pt", bufs=4) as ptp,
            tc.tile_pool(name="norm", bufs=1) as normp,
            tc.tile_pool(name="stg", bufs=2) as stgp,
            tc.tile_pool(name="ysb", bufs=3) as ypool,
            tc.tile_pool(name="psG", bufs=2, space="PSUM") as psG,
            tc.tile_pool(name="psS", bufs=2, space="PSUM") as psS,
            tc.tile_pool(name="psO", bufs=1, space="PSUM") as psO,
        ):
            # ---------------- constants / weights (scalar DMA queue) --------
            ones = constp.tile([1, 512], F32R, tag="ones")
            nc.scalar.dma_start(ones[:], ones_d[:])
            ones65 = constp.tile([65, 64], F32R, tag="ones65")
            nc.scalar.dma_start(ones65[:], ones65_d[:])
            bs_sb = constp.tile([128, NKB], F32, tag="bs")
            nc.scalar.dma_start(bs_sb[:], bs_d[:])

            wqk_sb = wts.tile([128, NCH, 512], F32R, tag="wqk")
            nc.scalar.dma_start(wqk_sb[:], wqk_d.rearrange("(c p) n -> p c n", p=128))
            wv_sb = wts.tile([128, NCH, 256], F32R, tag="wv")
            nc.scalar.dma_start(wv_sb[:], wv_d.rearrange("(c p) n -> p c n", p=128))
            wo_sb = wts.tile([128, 2, D], F32R, tag="wo")
            nc.scalar.dma_start(wo_sb[:], wo_d.rearrange("(c p) n -> p c n", p=128))
            bqk_sb = wts.tile([1, 512], F32R, tag="bqk")
            nc.scalar.dma_start(bqk_sb[:], bqk_d[:])
            bv_sb = wts.tile([1, 256], F32R, tag="bv")
            nc.scalar.dma_start(bv_sb[:], bv_d[:])

            # x^T chunks on the sync queue so they land first
            xT = bigp.tile([128, NCH, T], F32R, tag="xT")
            xt_chunks = xt_d.rearrange("(c p) t -> c p t", p=128)
            for tb in range(4):
                for c in range(NCH):
                    nc.sync.dma_start(
                        xT[:, c, tb * 512:(tb + 1) * 512],
                        xt_chunks[c][:, tb * 512:(tb + 1) * 512],
                    )

            qkt = [
                bigp.tile([128, T], BF16, tag=f"qkt{db}", name=f"qkt{db}")
                for db in range(4)
            ]
            vsb = bigp.tile([128, NKB, HPC, 65], BF16, tag="vsb")
            nc.vector.memset(vsb[:], 1.0)
            ot = [
                bigp.tile([128, T], F32R, tag=f"ot{pi}", name=f"ot{pi}")
                for pi in range(2)
            ]
            y_rows = y_d.rearrange("(n p) d -> n p d", p=128)

            # ---------------- emission helpers ----------------
            def emit_A(tb):
                for db in range(4):
                    ps = psG.tile([128, 512], F32, tag="gp", name=f"qk{tb}_{db}")
                    for c in range(NCH):
                        nc.tensor.matmul(
                            ps[:],
                            wqk_sb[:, c, db * 128:(db + 1) * 128],
                            xT[:, c, tb * 512:(tb + 1) * 512],
                            start=(c == 0),
                            stop=False,
                        )
                    nc.tensor.matmul(
                        ps[:],
                        bqk_sb[0:1, db * 128:(db + 1) * 128],
                        ones[0:1, :],
                        start=False,
                        stop=True,
                    )
                    nc.vector.tensor_copy(qkt[db][:, tb * 512:(tb + 1) * 512], ps[:])

            def emit_B(tb):
                for j in range(4):
                    kb = tb * 4 + j
                    ps = psG.tile([128, 256], F32, tag="gp", name=f"v{kb}")
                    for c in range(NCH):
                        nc.tensor.matmul(
                            ps[:],
                            xT[:, c, kb * 128:(kb + 1) * 128],
                            wv_sb[:, c, :],
                            start=(c == 0),
                            stop=False,
                        )
                    nc.tensor.matmul(
                        ps[:], ones[0:1, 0:128], bv_sb[:], start=False, stop=True
                    )
                    nc.vector.tensor_copy(
                        vsb[:, kb, :, 0:64],
                        ps[:].rearrange("p (h c) -> p h c", h=HPC),
                    )

            def emit_S_exp(qb, pi, kb):
                qdb, kdb = pi, 2 + pi
                s01 = psS.tile([128, 1024], F32, tag="s01", name=f"s{qb}_{pi}_{kb}")
                nc.tensor.matmul(
                    s01[:, 0:512],
                    qkt[kdb][0:64, kb * 128:(kb + 1) * 128],
                    qkt[qdb][0:64, qb * 512:(qb + 1) * 512],
                )
                nc.tensor.matmul(
                    s01[:, 512:1024],
                    qkt[kdb][64:128, kb * 128:(kb + 1) * 128],
                    qkt[qdb][64:128, qb * 512:(qb + 1) * 512],
                )
                p01 = ptp.tile([128, 1024], BF16, tag="p01", name=f"p{qb}_{pi}_{kb}")
                nc.scalar.activation(p01[:], s01[:], EXP, bias=bs_sb[:, kb:kb + 1])
                return p01

            def emit_D(qb, pi, kb, p01, osA, osB):
                nc.tensor.matmul(
                    osA[:], vsb[:, kb, 2 * pi, :], p01[:, 0:512],
                    start=(kb == 0), stop=(kb == NKB - 1),
                )
                nc.tensor.matmul(
                    osB[:], vsb[:, kb, 2 * pi + 1, :], p01[:, 512:1024],
                    start=(kb == 0), stop=(kb == NKB - 1),
                )

            def emit_post(qb, pi, osA, osB):
                # stage O' out of PSUM fast, then normalize from the staging copy
                for parity, os_ps in ((0, osA), (1, osB)):
                    stg = stgp.tile(
                        [65, 512], F32, tag=f"stg{parity}", name=f"stg{qb}_{pi}_{parity}"
                    )
                    nc.vector.tensor_copy(stg[:], os_ps[:])
                    # 1/denom on ScalarE as exp(-ln(d)): both functions live
                    # in the natural_log_exp_and_others table set (shared with
                    # the softmax exp -> no table switches), partition-local,
                    # and it slots into ACT's boundary idle instead of
                    # serializing the PE stream.
                    lnd = normp.tile([65, 512], F32, tag=f"lnd{parity}")
                    nc.scalar.activation(
                        lnd[64:65, :], stg[64:65, :],
                        mybir.ActivationFunctionType.Ln,
                    )
                    rec = normp.tile([65, 512], F32R, tag=f"rec{parity}")
                    nc.scalar.activation(
                        rec[64:65, :], lnd[64:65, :], EXP, scale=-1.0
                    )
                    bc_ps = psG.tile(
                        [64, 512], F32, tag="gp", name=f"bc{qb}_{pi}_{parity}"
                    )
                    nc.tensor.matmul(bc_ps[:], ones65[64:65, 0:64], rec[64:65, :])
                    cols = slice(qb * 512, (qb + 1) * 512)
                    if parity == 0:
                        nc.vector.tensor_mul(
                            ot[pi][0:64, cols], stg[0:64, :], bc_ps[0:64, :]
                        )
                    else:
                        stag = normp.tile([64, 512], F32R, tag="stag")
                        nc.vector.tensor_mul(stag[:], stg[0:64, :], bc_ps[0:64, :])
                        nc.sync.dma_start(ot[pi][64:128, cols], stag[:])

            def make_E_chunks(qb):
                # Stage E for one q-block, sliced into 16 small closures so the
                # emission can interleave one chunk per k-iteration of the next
                # attention group (keeps PE dense without starving ScalarE).
                chunks = []
                for j in range(4):
                    tb = qb * 4 + j
                    state = {}

                    def c0(tb=tb, state=state):
                        state["ysb"] = ypool.tile(
                            [128, D], F32, tag="ysb", name=f"ysb{tb}"
                        )
                        state["yps"] = [
                            psG.tile([128, 512], F32, tag="gp", name=f"yps{tb}_{nb}")
                            for nb in range(2)
                        ]
                        nc.tensor.matmul(
                            state["yps"][0][:],
                            ot[0][:, tb * 128:(tb + 1) * 128],
                            wo_sb[:, 0, 0:512],
                            start=True, stop=False,
                        )

                    def c1(tb=tb, state=state):
                        nc.tensor.matmul(
                            state["yps"][1][:],
                            ot[0][:, tb * 128:(tb + 1) * 128],
                            wo_sb[:, 0, 512:1024],
                            start=True, stop=False,
                        )

                    def c2(tb=tb, state=state):
                        nc.tensor.matmul(
                            state["yps"][0][:],
                            ot[1][:, tb * 128:(tb + 1) * 128],
                            wo_sb[:, 1, 0:512],
                            start=False, stop=True,
                        )
                        nc.vector.tensor_copy(
                            state["ysb"][:, 0:512], state["yps"][0][:]
                        )

                    def c3(tb=tb, state=state):
                        nc.tensor.matmul(
                            state["yps"][1][:],
                            ot[1][:, tb * 128:(tb + 1) * 128],
                            wo_sb[:, 1, 512:1024],
                            start=False, stop=True,
                        )
                        nc.vector.tensor_copy(
                            state["ysb"][:, 512:1024], state["yps"][1][:]
                        )
                        nc.sync.dma_start(y_rows[tb], state["ysb"][:])

                    chunks += [c0, c1, c2, c3]
                return chunks

            def emit_E(qb):
                for ch in make_E_chunks(qb):
                    ch()

            # ---------------- pipelined emission ----------------
            # Phase 1: stages A/B per t-group, with CD(q0, pair0) k-iterations
            # interleaved so ScalarE ramps while the PE grinds projections.
            osA = psO.tile([65, 512], F32, tag="osA", name="osA0_0")
            osB = psO.tile([65, 512], F32, tag="osB", name="osB0_0")
            for tb in range(4):
                emit_A(tb)
                emit_B(tb)
                for kb in range(4 * tb, 4 * tb + 4):
                    p01 = emit_S_exp(0, 0, kb)
                    emit_D(0, 0, kb, p01, osA, osB)
            pending = [(0, 0, osA, osB)]

            # Phase 2: remaining groups; each group's first two S/exp pairs
            # are emitted before the previous group's epilogue so ACT stays fed
            # across the boundary.
            groups = [(0, 1)] + [(qb, pi) for qb in range(1, NQB) for pi in range(2)]
            e_chunks = []
            for qb, pi in groups:
                head = [emit_S_exp(qb, pi, kb) for kb in (0, 1)]
                # previous group's epilogue; its E work is sliced into chunks
                # and interleaved below
                pqb, ppi, posA, posB = pending.pop()
                emit_post(pqb, ppi, posA, posB)
                if ppi == 1:
                    e_chunks = make_E_chunks(pqb)
                osA = psO.tile([65, 512], F32, tag="osA", name=f"osA{qb}_{pi}")
                osB = psO.tile([65, 512], F32, tag="osB", name=f"osB{qb}_{pi}")
                for kb in (0, 1):
                    emit_D(qb, pi, kb, head[kb], osA, osB)
                for kb in range(2, NKB):
                    p01 = emit_S_exp(qb, pi, kb)
                    emit_D(qb, pi, kb, p01, osA, osB)
                    if e_chunks:
                        e_chunks.pop(0)()
                pending = [(qb, pi, osA, osB)]
                while e_chunks:
                    e_chunks.pop(0)()

            qb, pi, osA, osB = pending.pop()
            emit_post(qb, pi, osA, osB)
            emit_E(qb)

    nc.compile()
    return nc


def _get_nc():
    if "nc" not in _NC_CACHE:
        _NC_CACHE["nc"] = _build_nc()
    return _NC_CACHE["nc"]


def _make_in_maps(x, boundary_score, W_qkv, b_qkv, W_out):
    x = np.asarray(x, np.float32)
    boundary_score = np.asarray(boundary_score, np.float32)
    W_qkv = np.asarray(W_qkv, np.float32)
    b_qkv = np.asarray(b_qkv, np.float32)
    W_out = np.asarray(W_out, np.float32)

    Wq, Wk, Wv = W_qkv[:, :D], W_qkv[:, D:2 * D], W_qkv[:, 2 * D:]
    bq, bk, bv = b_qkv[:D], b_qkv[D:2 * D], b_qkv[2 * D:]
    ones = np.ones((1, 512), np.float32)
    ones65 = np.ones((65, 64), np.float32)
    xts = [np.ascontiguousarray(x[b].T) for b in range(x.shape[0])]

    in_maps = []
    for c in range(8):
        b, g = divmod(c, 4)
        lo, hi = 256 * g, 256 * (g + 1)
        wqk = np.ascontiguousarray(
            np.concatenate([Wq[:, lo:hi] * SCALE, Wk[:, lo:hi]], axis=1)
        )
        bqk = np.concatenate([bq[lo:hi] * SCALE, bk[lo:hi]])[None]
        wv = np.ascontiguousarray(Wv[:, lo:hi])
        bvv = np.ascontiguousarray(bv[lo:hi][None])
        wo = np.ascontiguousarray(W_out[lo:hi, :])
        bs = np.ascontiguousarray(
            (boundary_score[b] * BIAS_COEF).reshape(NKB, 128).T
        )
        in_maps.append(
            dict(
                xt=xts[b], wqk=wqk, bqk=np.ascontiguousarray(bqk),
                wv=wv, bv=bvv, wo=wo, bs=bs, ones=ones, ones65=ones65,
            )
        )
    return in_maps


def kernel(x, boundary_score, W_qkv, b_qkv, W_out, b_out):
    from concourse.bass_utils import run_bass_kernel_spmd

    x = np.asarray(x, np.float32)
    B = x.shape[0]
    in_maps = _make_in_maps(x, boundary_score, W_qkv, b_qkv, W_out)
    nc = _get_nc()
    res = run_bass_kernel_spmd(nc, in_maps, list(range(8))).results
    out = np.zeros((B, T, D), np.float32)
    for c in range(8):
        out[c // 4] += res[c]["y"]
    out += np.asarray(b_out, np.float32)
    return out
